# revision 3
# baseline (speedup 1.0000x reference)
"""Trainium2 Bass kernel for nn_Attention_9560597201123.

Full multi-head attention (B=4, N=2048, E=1024, H=16, D=64), f32 reference.

Sharding: 8 cores = (batch b in 0..4) x (sequence half in 0..2). Each core:
  - receives x[b].T (query-half columns first), full qkv/proj weights
  - computes k/v projections for the FULL batch-b sequence (2x redundant vs
    an exchange, but avoids slow 2-rank collectives entirely)
  - computes q projection + attention + output projection for its 1024
    query rows, returns y [1024, 1024]
Host assembles the 8 shards into [4, 2048, 1024].

Device layout notes:
  - scores are computed transposed (S^T: keys on partitions, queries free)
    so that P^T = exp(S^T) feeds the PV matmul directly (contraction = keys)
  - softmax normalizer: v is extended with a ones column (lhsT M=65), so
    the PV matmul's partition 64 accumulates the row sums for free
  - all TensorEngine matmuls run in bf16 (1 cycle/row); accumulation f32
"""

import numpy as np
import ml_dtypes

P = 128
SEQ = 2048
QH = 1024  # queries per core
E = 1024
H = 16
NPAIR = 8  # head pairs
D = 64
KC = 16  # key chunks of 128
EC = 8  # e_in chunks of 128
SCALE = D ** -0.5  # 0.125

_NC = None


def build_nc():
    global _NC
    if _NC is not None:
        return _NC

    import concourse.bass as bass  # noqa: F401
    import concourse.mybir as mybir
    import concourse.tile as tile
    from concourse import bacc

    BF = mybir.dt.bfloat16
    F32 = mybir.dt.float32
    EXP = mybir.ActivationFunctionType.Exp
    ADD = mybir.AluOpType.add
    MULT = mybir.AluOpType.mult

    nc = bacc.Bacc("TRN2", target_bir_lowering=False, debug=False, num_devices=8)

    xt_d = nc.dram_tensor("xt", [E, SEQ], BF, kind="ExternalInput").ap()
    wqkv_d = nc.dram_tensor("wqkv", [E, 3 * E], BF, kind="ExternalInput").ap()
    bqkv_d = nc.dram_tensor("bqkv", [3 * E], F32, kind="ExternalInput").ap()
    wp_d = nc.dram_tensor("wp", [E, E], BF, kind="ExternalInput").ap()
    bv_bf_d = nc.dram_tensor("bv_bf", [E], BF, kind="ExternalInput").ap()
    bp_bf_d = nc.dram_tensor("bp_bf", [E], BF, kind="ExternalInput").ap()
    out_d = nc.dram_tensor("out", [QH, E], F32, kind="ExternalOutput").ap()

    wqkv_r = wqkv_d.rearrange("(o p) c -> p o c", p=P)

    with tile.TileContext(nc) as tc:
        with (
            tc.tile_pool(name="persist", bufs=1) as persist,
            tc.tile_pool(name="wstream", bufs=2) as wstream,
            tc.tile_pool(name="ptpool", bufs=3) as ptpool,
            tc.tile_pool(name="small", bufs=2) as small,
            tc.tile_pool(name="proj_ps", bufs=2, space="PSUM") as proj_ps,
            tc.tile_pool(name="acc_ps", bufs=2, space="PSUM") as acc_ps,
            tc.tile_pool(name="sc_ps", bufs=2, space="PSUM") as sc_ps,
        ):
            # ---- persistent tiles + input DMA ----
            xt = persist.tile([P, EC, SEQ], BF, tag="xt")
            nc.sync.dma_start(xt[:], xt_d.rearrange("(o p) s -> p o s", p=P))

            vx = persist.tile([P, KC, H * 65], BF, tag="vx")
            vx4 = vx.rearrange("p s (h c) -> p s h c", c=65)
            nc.vector.memset(vx4[:, :, :, 64], 1.0)

            kt = persist.tile([P, NPAIR, SEQ], BF, tag="kt")
            qt = persist.tile([P, NPAIR, QH], BF, tag="qt")
            aT = persist.tile([P, NPAIR, QH], BF, tag="aT")

            bq_k = persist.tile([P, EC], F32, tag="bq_k")
            nc.sync.dma_start(bq_k[:], bqkv_d[E : 2 * E].rearrange("(o p) -> p o", p=P))
            bq_q = persist.tile([P, EC], F32, tag="bq_q")
            nc.sync.dma_start(bq_q[:], bqkv_d[0:E].rearrange("(o p) -> p o", p=P))

            bv_row = persist.tile([1, E], BF, tag="bv_row")
            nc.sync.dma_start(bv_row[:], bv_bf_d[None])
            bv_bc = persist.tile([P, E], BF, tag="bv_bc")
            nc.gpsimd.partition_broadcast(bv_bc[:], bv_row[:])

            pb_row = persist.tile([1, E], BF, tag="pb_row")
            nc.sync.dma_start(pb_row[:], bp_bf_d[None])
            pb_bc = persist.tile([P, E], BF, tag="pb_bc")
            nc.gpsimd.partition_broadcast(pb_bc[:], pb_row[:])

            # ---- phase V: v projection (full sequence), into vx (65-strided) ----
            with tc.tile_pool(name="wvpool", bufs=1) as wvpool:
                wv = wvpool.tile([P, EC, E], BF, tag="wv")
                nc.sync.dma_start(wv[:], wqkv_r[:, :, 2 * E : 3 * E])
                for sm in range(KC):
                    for vc in range(2):
                        ps = proj_ps.tile([P, 512], mybir.dt.float32, tag="ps512")
                        for ec in range(EC):
                            nc.tensor.matmul(
                                ps[:],
                                lhsT=xt[:, ec, sm * P : (sm + 1) * P],
                                rhs=wv[:, ec, vc * 512 : (vc + 1) * 512],
                                start=(ec == 0),
                                stop=(ec == EC - 1),
                            )
                        nc.vector.tensor_tensor(
                            out=vx4[:, sm, vc * 8 : (vc + 1) * 8, 0:64],
                            in0=ps[:].rearrange("p (h c) -> p h c", c=64),
                            in1=bv_bc[:, vc * 512 : (vc + 1) * 512].rearrange(
                                "p (h c) -> p h c", c=64
                            ),
                            op=ADD,
                        )

            # ---- per pair: k/q projections then attention ----
            for p in range(NPAIR):
                wk = wstream.tile([P, EC, P], BF, tag="wk")
                nc.sync.dma_start(wk[:], wqkv_r[:, :, E + p * P : E + (p + 1) * P])
                for s in range(4):
                    ps = proj_ps.tile([P, 512], mybir.dt.float32, tag="ps512")
                    for ec in range(EC):
                        nc.tensor.matmul(
                            ps[:],
                            lhsT=wk[:, ec, :],
                            rhs=xt[:, ec, s * 512 : (s + 1) * 512],
                            start=(ec == 0),
                            stop=(ec == EC - 1),
                        )
                    nc.vector.tensor_scalar_add(
                        out=kt[:, p, s * 512 : (s + 1) * 512],
                        in0=ps[:],
                        scalar1=bq_k[:, p : p + 1],
                    )

                wq = wstream.tile([P, EC, P], BF, tag="wq")
                nc.sync.dma_start(wq[:], wqkv_r[:, :, p * P : (p + 1) * P])
                for s in range(2):
                    ps = proj_ps.tile([P, 512], mybir.dt.float32, tag="ps512")
                    for ec in range(EC):
                        nc.tensor.matmul(
                            ps[:],
                            lhsT=wq[:, ec, :],
                            rhs=xt[:, ec, s * 512 : (s + 1) * 512],
                            start=(ec == 0),
                            stop=(ec == EC - 1),
                        )
                    nc.vector.tensor_scalar_add(
                        out=qt[:, p, s * 512 : (s + 1) * 512],
                        in0=ps[:],
                        scalar1=bq_q[:, p : p + 1],
                    )

                # attention for head pair p (heads 2p, 2p+1)
                for qb in range(2):
                    qsl = slice(qb * 512, (qb + 1) * 512)
                    accA = acc_ps.tile([65, 512], mybir.dt.float32, tag="acc")
                    accB = acc_ps.tile([65, 512], mybir.dt.float32, tag="acc")
                    for kc in range(KC):
                        sc = sc_ps.tile([P, 1024], mybir.dt.float32, tag="sc")
                        nc.tensor.matmul(
                            sc[:, 0:512],
                            lhsT=kt[0:64, p, kc * P : (kc + 1) * P],
                            rhs=qt[0:64, p, qsl],
                        )
                        nc.tensor.matmul(
                            sc[:, 512:1024],
                            lhsT=kt[64:P, p, kc * P : (kc + 1) * P],
                            rhs=qt[64:P, p, qsl],
                        )
                        pt = ptpool.tile([P, 1024], BF, tag="pt")
                        nc.scalar.activation(out=pt[:], in_=sc[:], func=EXP, scale=SCALE)
                        nc.tensor.matmul(
                            accA[:],
                            lhsT=vx4[:, kc, 2 * p, :],
                            rhs=pt[:, 0:512],
                            start=(kc == 0),
                            stop=(kc == KC - 1),
                        )
                        nc.tensor.matmul(
                            accB[:],
                            lhsT=vx4[:, kc, 2 * p + 1, :],
                            rhs=pt[:, 512:1024],
                            start=(kc == 0),
                            stop=(kc == KC - 1),
                        )
                    # normalize by row sums (partition 64 of acc) and store aT
                    for hh, acc in ((0, accA), (1, accB)):
                        rs = small.tile([1, 512], F32, tag="rs")
                        nc.vector.reciprocal(rs[:], acc[64:65, :])
                        R = small.tile([64, 512], F32, tag="R")
                        nc.gpsimd.partition_broadcast(R[:], rs[:])
                        if hh == 0:
                            nc.vector.tensor_tensor(
                                out=aT[0:64, p, qsl],
                                in0=acc[0:64, :],
                                in1=R[:],
                                op=MULT,
                            )
                        else:
                            tmpb = small.tile([64, 512], BF, tag="tmpb")
                            nc.vector.tensor_tensor(
                                out=tmpb[:], in0=acc[0:64, :], in1=R[:], op=MULT
                            )
                            nc.sync.dma_start(aT[64:P, p, qsl], tmpb[:])

            # ---- output projection ----
            with tc.tile_pool(name="pwpool", bufs=1) as pwpool:
                pw = pwpool.tile([P, NPAIR, E], BF, tag="pw")
                nc.sync.dma_start(pw[:], wp_d.rearrange("(o p) c -> p o c", p=P))
                for qc in range(8):
                    for ncol in range(2):
                        nsl = slice(ncol * 512, (ncol + 1) * 512)
                        yps = proj_ps.tile([P, 512], mybir.dt.float32, tag="ps512")
                        for p in range(NPAIR):
                            nc.tensor.matmul(
                                yps[:],
                                lhsT=aT[:, p, qc * P : (qc + 1) * P],
                                rhs=pw[:, p, nsl],
                                start=(p == 0),
                                stop=(p == NPAIR - 1),
                            )
                        ysb = small.tile([P, 512], F32, tag="ysb")
                        nc.vector.tensor_tensor(
                            out=ysb[:], in0=yps[:], in1=pb_bc[:, nsl], op=ADD
                        )
                        nc.sync.dma_start(out_d[qc * P : (qc + 1) * P, nsl], ysb[:])

    nc.finalize()
    _NC = nc
    return nc


def make_in_maps(x, qkv_w, qkv_b, proj_w, proj_b):
    bf16 = ml_dtypes.bfloat16
    x = np.asarray(x, dtype=np.float32)
    wqkv = np.ascontiguousarray(np.asarray(qkv_w, dtype=np.float32)).astype(bf16)
    bqkv = np.ascontiguousarray(np.asarray(qkv_b, dtype=np.float32))
    wp = np.ascontiguousarray(np.asarray(proj_w, dtype=np.float32)).astype(bf16)
    bp = np.ascontiguousarray(np.asarray(proj_b, dtype=np.float32))
    in_maps = []
    for c in range(8):
        b, half = divmod(c, 2)
        xt = x[b].T  # [E, SEQ]
        if half == 0:
            xperm = xt
        else:
            xperm = np.concatenate([xt[:, QH:], xt[:, :QH]], axis=1)
        in_maps.append(
            {
                "xt": np.ascontiguousarray(xperm).astype(bf16),
                "wqkv": wqkv,
                "bqkv": bqkv,
                "wp": wp,
                "bv_bf": bqkv[2 * E : 3 * E].astype(bf16),
                "bp_bf": bp.astype(bf16),
            }
        )
    return in_maps


def assemble_out(results):
    out = np.empty((4, SEQ, E), dtype=np.float32)
    for c in range(8):
        b, half = divmod(c, 2)
        out[b, half * QH : (half + 1) * QH, :] = results[c]["out"]
    return out


def run(inputs, trace=False):
    """Run on 8 NeuronCores; returns (output, BassKernelResults)."""
    from concourse.bass_utils import run_bass_kernel_spmd

    nc = build_nc()
    in_maps = make_in_maps(**inputs)
    res = run_bass_kernel_spmd(nc, in_maps, core_ids=list(range(8)), trace=trace)
    return assemble_out(res.results), res


def kernel(x, qkv_w, qkv_b, proj_w, proj_b):
    out, _ = run(
        dict(x=x, qkv_w=qkv_w, qkv_b=qkv_b, proj_w=proj_w, proj_b=proj_b),
        trace=False,
    )
    return out


if __name__ == "__main__":
    rng = np.random.default_rng(0)
    x = rng.standard_normal((4, SEQ, E), dtype=np.float32)
    s = E ** -0.5
    inputs = dict(
        x=x,
        qkv_w=rng.standard_normal((E, 3 * E), dtype=np.float32) * s,
        qkv_b=rng.standard_normal((3 * E,), dtype=np.float32) * 0.02,
        proj_w=rng.standard_normal((E, E), dtype=np.float32) * s,
        proj_b=rng.standard_normal((E,), dtype=np.float32) * 0.02,
    )
    out = kernel(**inputs)
    print("out", out.shape, out.dtype, float(np.abs(out).mean()))
